# revision 21
# baseline (speedup 1.0000x reference)
"""Distributed Bass kernel: LN + multi-head ALiBi attention + out-proj (v3).

Sharding: 8 cores = (batch b in 0..3) x (query-token half t in 0..1).
Each core computes the full pipeline for its 1024 query tokens (all 16
heads); K/V are computed for the 2048-token sequence (duplicated across
the 2 cores of a batch), restricted to the union of the heads' ALiBi
bands.  No collectives.

SPMD trick: every core runs the SAME graph.  Core (b, t) receives x[b]
rolled by -1024*t along tokens, so its query tokens always sit at local
rows 0..1023.  Distance tables (master/master1) are per-core DATA.

v3 structure (vs the W=512 baseline):
  * ALiBi banding: head h attends within |i-j| <= T_h = ceil(19/s_h).
    Heads 0..13 (T<=121) use diagonal-aligned [128q x 3*128j] blocks:
    query chunk c attends j-tiles {c-1, c, c+1} (mod 16).  This halves
    score/bias/exp/PV volume vs 512-wide blocks, and the bias slice for
    a chunk is ONE contiguous master window (identical for all chunks).
  * K/V projections skip (head-group, j-tile) units outside the bands;
    projection loops reuse each loaded weight for 2+ matmuls.
  * Head pairs (2k, 2k+1) occupy PE row groups [0:64]/[64:128]; their
    k=64 score matmuls run concurrently (row tiling) and LDWEIGHTS of
    one parity overlaps the other parity's matmul.
  * Score bias is added in PSUM in place (DVE) and exp reads PSUM.
  * Softmax denominators ride the PV matmul as a ones-column (row 64),
    are collected into lrows[16,512] per q-half, inverted once with
    DVE reciprocal, and broadcast back with a one-hot matmul (no Ln/
    exp(-x) chain, no ACT table thrashing).
  * Emission interleaves late projection work into the attention stream
    so the PE never idles long enough to re-throttle (HAM).
"""

import os
import sys

sys.path.insert(0, "/opt/trn_rl_repo")

import numpy as np
import ml_dtypes

import concourse.bass as bass
import concourse.mybir as mybir
import concourse.tile as tile
from concourse import bacc
from concourse.bass import ts
from concourse.bass_utils import run_bass_kernel_spmd

BF16 = mybir.dt.bfloat16
F32 = mybir.dt.float32
F32R = mybir.dt.float32r

CTX = 2048
DIM = 1024
NH = 16
DH = 64
QTOK = 1024
EPS = 1e-5
MTRIM = 1152  # master table column trim (cols < 1152 never read)
MW = 3072 - MTRIM

LAST_EXEC_NS = None


def _slotX(jt):
    """xnT/KT/Vsb packed slot order: [0..8, 15, 9..14]."""
    if jt <= 8:
        return jt
    if jt == 15:
        return 9
    return jt + 1


def _band_blocks(T, qc):
    """W=512 j-tile list for query chunk qc (g0 = qc*512), half-width T."""
    g0 = qc * 512
    lo = max(0, g0 - T) // 128
    hi = (min(CTX, g0 + 512 + T) + 127) // 128
    jts = set(range(lo, hi))
    if qc == 0 and T < CTX:
        jts |= set(range((CTX - T) // 128, CTX // 128))
    return sorted(jts)


def _build_graph(s_heads, Ts):
    nc = bacc.Bacc("TRN2", target_bir_lowering=False, debug=False)

    x_d = nc.dram_tensor("x", [CTX, DIM], BF16, kind="ExternalInput").ap()
    wq_d = nc.dram_tensor("wq", [8, 128, 8, 128], BF16, kind="ExternalInput").ap()
    wk_d = nc.dram_tensor("wk", [8, 128, 8, 128], BF16, kind="ExternalInput").ap()
    wv_d = nc.dram_tensor("wv", [8, 128, DIM], BF16, kind="ExternalInput").ap()
    wo_d = nc.dram_tensor("wo", [8, 128, DIM], BF16, kind="ExternalInput").ap()
    mst_d = nc.dram_tensor("master", [128, MW], BF16, kind="ExternalInput").ap()
    mst1_d = nc.dram_tensor("master1", [128, 2048], BF16, kind="ExternalInput").ap()
    idn_d = nc.dram_tensor("ident", [128, 128], BF16, kind="ExternalInput").ap()
    oh_d = nc.dram_tensor("oh16", [16, 2048], F32R, kind="ExternalInput").ap()
    out_d = nc.dram_tensor("out", [QTOK, DIM], F32, kind="ExternalOutput").ap()

    AF = mybir.ActivationFunctionType
    ALU = mybir.AluOpType

    KTW = 7 * 1280 + 2048  # banded KT: 10 slots for dqt<7, 16 for dqt7

    def kt_col(dqt, jt):
        return min(dqt, 7) * 1280 + 128 * _slotX(jt)

    with tile.TileContext(nc) as tc:
        with (
            tc.tile_pool(name="persist", bufs=1) as pp,
            tc.tile_pool(name="xio", bufs=5) as xp,
            tc.tile_pool(name="xnp", bufs=4) as xnp,
            tc.tile_pool(name="wstream", bufs=3) as wp,
            tc.tile_pool(name="ptp", bufs=4) as ptp,
            tc.tile_pool(name="small", bufs=4) as sp,
            tc.tile_pool(name="lpool", bufs=2) as lp,
            tc.tile_pool(name="ltp", bufs=2) as ltp,
            tc.tile_pool(name="epool", bufs=2) as epool,
            tc.tile_pool(name="wopool", bufs=1) as wop,
            tc.tile_pool(name="opool", bufs=2) as op,
            tc.tile_pool(name="psA", bufs=2, space="PSUM") as psA,
            tc.tile_pool(name="psC", bufs=2, space="PSUM") as psC,
            tc.tile_pool(name="pso", bufs=2, space="PSUM") as pso,
        ):
            # ---- persistent SBUF ----
            master = pp.tile([128, MW], BF16, tag="master")
            master1 = pp.tile([128, 2048], BF16, tag="master1")
            ident = pp.tile([128, 128], BF16, tag="ident")
            oh16 = pp.tile([16, 2048], F32R, tag="oh16")
            xnT = pp.tile([128, 8, CTX], BF16, tag="xnT")
            KT = pp.tile([128, KTW], BF16, tag="KT")
            QT = pp.tile([128, 8, QTOK], BF16, tag="QT")
            VsbA = pp.tile([128, 10, NH, 65], BF16, tag="VsbA")
            VsbB = pp.tile([128, 6, 2, 65], BF16, tag="VsbB")
            OT = pp.tile([128, 8, QTOK], BF16, tag="OT")
            wv_sb = pp.tile([128, 8, DIM], BF16, tag="wv")
            eps_sb = pp.tile([128, 1], F32, tag="eps")

            nc.scalar.dma_start(master[:], mst_d[:])
            nc.scalar.dma_start(master1[:], mst1_d[:])
            nc.scalar.dma_start(ident[:], idn_d[:])
            nc.scalar.dma_start(oh16[:], oh_d[:])
            nc.scalar.dma_start(wv_sb[:], wv_d.rearrange("k p d -> p k d"))
            nc.any.memset(eps_sb[:], EPS)
            nc.any.memset(VsbA[:, :, :, 64:65], 1.0)
            nc.any.memset(VsbB[:, :, :, 64:65], 1.0)

            def vsb_slice(jt, h):
                if jt in (9, 10, 11, 12, 13, 14):
                    assert h >= 14
                    return VsbB[:, jt - 9, h - 14, 0:65]
                return VsbA[:, _slotX(jt), h, 0:65]

            def msrc(jt, off):
                """(tile, col) for a W=512 bias slice at original col off."""
                if jt >= 8:
                    return master1, off
                return master, off - MTRIM

            # ---------- LN + transpose for one 512-token chunk ----------
            def emit_chunk(ch):
                mv4 = sp.tile([128, 4, 2], F32, tag="mv4")
                xts = []
                for i, tt in enumerate(range(4 * ch, 4 * ch + 4)):
                    xt = xp.tile([128, DIM], BF16, tag="xt")
                    nc.sync.dma_start(xt[:], x_d[ts(tt, 128), :])
                    xts.append(xt)
                    st6 = sp.tile([128, 2, 6], F32, tag="st6")
                    nc.vector.bn_stats(st6[:, 0, :], xt[:, 0:512])
                    nc.vector.bn_stats(st6[:, 1, :], xt[:, 512:1024])
                    nc.vector.bn_aggr(mv4[:, i, :], st6[:])
                lv4 = sp.tile([128, 4], F32, tag="lv4")
                nc.scalar.activation(lv4[:], mv4[:, :, 1], AF.Ln, bias=eps_sb[:])
                rs4 = sp.tile([128, 4], F32, tag="rs4")
                nc.scalar.activation(rs4[:], lv4[:], AF.Exp, scale=-0.5)
                xns = []
                for i in range(4):
                    xn = xnp.tile([128, DIM], BF16, tag="xn")
                    nc.vector.tensor_scalar(
                        xn[:], xts[i][:], mv4[:, i, 0:1], rs4[:, i:i + 1],
                        ALU.subtract, ALU.mult,
                    )
                    xns.append(xn)
                # transpose on PE -> xnT packed slots
                jts = list(range(4 * ch, 4 * ch + 4))
                for ko in range(8):
                    tp = psC.tile([128, 512], F32, tag="psC")
                    tpv = tp[:].bitcast(BF16)
                    for i in range(4):
                        nc.tensor.transpose(
                            tpv[:, ts(i, 128)], xns[i][:, ts(ko, 128)], ident[:]
                        )
                    # evacuate to packed slots (contiguous runs)
                    runs = []
                    start = 0
                    while start < 4:
                        end = start
                        while (end + 1 < 4 and
                               _slotX(jts[end + 1]) == _slotX(jts[end]) + 1):
                            end += 1
                        runs.append((start, end))
                        start = end + 1
                    for (a, b) in runs:
                        w = 128 * (b - a + 1)
                        c0 = 128 * _slotX(jts[a])
                        nc.vector.tensor_copy(
                            xnT[:, ko, c0:c0 + w], tpv[:, 128 * a:128 * a + w]
                        )

            # ---------- projection helpers ----------
            def emit_k_waveA():
                """K projection over chunks 0,1 (xnT cols 0:1024)."""
                for dqt in range(8):
                    wt = wp.tile([128, 8, 128], BF16, tag="wt")
                    nc.scalar.dma_start(wt[:], wk_d[dqt])
                    ps = psA.tile([128, 1024], F32, tag="psA")
                    for ko in range(8):
                        nc.tensor.matmul(
                            ps[:, 0:512], wt[:, ko, :], xnT[:, ko, 0:512],
                            start=(ko == 0), stop=(ko == 7),
                        )
                        nc.tensor.matmul(
                            ps[:, 512:1024], wt[:, ko, :], xnT[:, ko, 512:1024],
                            start=(ko == 0), stop=(ko == 7),
                        )
                    c0 = kt_col(dqt, 0)
                    nc.vector.tensor_copy(KT[:, c0:c0 + 1024], ps[:])

            def emit_q_half(dqt, qh):
                """Q projection for one dqt, one 512-token query half."""
                wt = wp.tile([128, 8, 128], BF16, tag="wt")
                nc.scalar.dma_start(wt[:], wq_d[dqt])
                ps = psC.tile([128, 512], F32, tag="psC")
                for ko in range(8):
                    nc.tensor.matmul(
                        ps[:], wt[:, ko, :], xnT[:, ko, ts(qh, 512)],
                        start=(ko == 0), stop=(ko == 7),
                    )
                nc.vector.tensor_copy(QT[:, dqt, ts(qh, 512)], ps[:])

            def emit_k_band(tile_jt, dqts):
                """K^T band tile (jt 8 or 15) for a subset of dqt<=6."""
                sx = 128 * _slotX(tile_jt)
                for dqt in dqts:
                    wt = wp.tile([128, 8, 128], BF16, tag="wt")
                    nc.scalar.dma_start(wt[:], wk_d[dqt])
                    ps = psC.tile([128, 512], F32, tag="psC")
                    for ko in range(8):
                        nc.tensor.matmul(
                            ps[:, 0:128], wt[:, ko, :],
                            xnT[:, ko, sx:sx + 128],
                            start=(ko == 0), stop=(ko == 7),
                        )
                    c0 = kt_col(dqt, tile_jt)
                    nc.vector.tensor_copy(KT[:, c0:c0 + 128], ps[:, 0:128])

            def emit_k_dqt7_hi():
                """dqt7 K^T for xnT slots 8..15 (cols 1024:2048), psC pair."""
                wt = wp.tile([128, 8, 128], BF16, tag="wt")
                nc.scalar.dma_start(wt[:], wk_d[7])
                for half in range(2):
                    ps = psC.tile([128, 512], F32, tag="psC")
                    for ko in range(8):
                        nc.tensor.matmul(
                            ps[:], wt[:, ko, :],
                            xnT[:, ko, 1024 + 512 * half:1536 + 512 * half],
                            start=(ko == 0), stop=(ko == 7),
                        )
                    c0 = 7 * 1280 + 1024 + 512 * half
                    nc.vector.tensor_copy(KT[:, c0:c0 + 512], ps[:])

            def emit_v_full(jt, pool):
                """V proj for all 16 heads at j-tile jt (slot in VsbA)."""
                sx = 128 * _slotX(jt)
                if pool is psA:
                    ps = psA.tile([128, 1024], F32, tag="psA")
                    for ko in range(8):
                        nc.tensor.matmul(
                            ps[:, 0:512], xnT[:, ko, sx:sx + 128],
                            wv_sb[:, ko, 0:512],
                            start=(ko == 0), stop=(ko == 7),
                        )
                        nc.tensor.matmul(
                            ps[:, 512:1024], xnT[:, ko, sx:sx + 128],
                            wv_sb[:, ko, 512:1024],
                            start=(ko == 0), stop=(ko == 7),
                        )
                    nc.vector.tensor_copy(
                        VsbA[:, _slotX(jt), :, 0:64],
                        ps[:].rearrange("p (h d) -> p h d", d=64),
                    )
                else:
                    for half in range(2):
                        ps = psC.tile([128, 512], F32, tag="psC")
                        for ko in range(8):
                            nc.tensor.matmul(
                                ps[:], xnT[:, ko, sx:sx + 128],
                                wv_sb[:, ko, ts(half, 512)],
                                start=(ko == 0), stop=(ko == 7),
                            )
                        nc.vector.tensor_copy(
                            VsbA[:, _slotX(jt), 8 * half:8 * half + 8, 0:64],
                            ps[:].rearrange("p (h d) -> p h d", d=64),
                        )

            def emit_v_b(jts):
                """V proj for heads 14,15 only (VsbB), jts subset of 9..14."""
                for jt in jts:
                    sx = 128 * _slotX(jt)
                    ps = psC.tile([128, 512], F32, tag="psC")
                    for ko in range(8):
                        nc.tensor.matmul(
                            ps[:, 0:128], xnT[:, ko, sx:sx + 128],
                            wv_sb[:, ko, 896:1024],
                            start=(ko == 0), stop=(ko == 7),
                        )
                    nc.vector.tensor_copy(
                        VsbB[:, jt - 9, :, 0:64],
                        ps[:, 0:128].rearrange("p (h d) -> p h d", d=64),
                    )

            # ---------- attention: banded pair (heads 2k,2k+1), W=128 ----
            def emit_pair_small(dqt, qh, lrows, pump=None):
                po = {}
                for par in range(2):
                    po[par] = pso.tile([65, 512], F32, tag="po", name=f"po{par}")
                pts = []
                for cl in range(4):
                    c = 4 * qh + cl
                    slot = psA.tile([128, 1024], F32, tag="psA")
                    for k in range(3):
                        jt = (c + 1 - k) % 16
                        col = kt_col(dqt, jt)
                        for par in range(2):
                            b = 64 * par
                            nc.tensor.matmul(
                                slot[:, 512 * par + 128 * k:512 * par + 128 * k + 128],
                                KT[b:b + 64, col:col + 128],
                                QT[b:b + 64, dqt, 128 * c:128 * c + 128],
                                start=(k == 0), stop=(k == 2),
                            )
                    pt = ptp.tile([128, 1024], BF16, tag="pt")
                    for par in range(2):
                        b = 512 * par
                        if c == 0:
                            # k=0,1 from master, k=2 (wrap jt 15) from master1
                            nc.vector.tensor_tensor(
                                slot[:, b:b + 256], slot[:, b:b + 256],
                                master[:, 1920 - MTRIM:2176 - MTRIM], ALU.add,
                            )
                            nc.vector.tensor_tensor(
                                slot[:, b + 256:b + 384], slot[:, b + 256:b + 384],
                                master1[:, 128:256], ALU.add,
                            )
                        elif c == 7:
                            # k=0 (jt 8 crosses the roll-wrap plane) -> master1
                            nc.vector.tensor_tensor(
                                slot[:, b:b + 128], slot[:, b:b + 128],
                                master1[:, 1920:2048], ALU.add,
                            )
                            nc.vector.tensor_tensor(
                                slot[:, b + 128:b + 384], slot[:, b + 128:b + 384],
                                master[:, 2048 - MTRIM:2304 - MTRIM], ALU.add,
                            )
                        else:
                            nc.vector.tensor_tensor(
                                slot[:, b:b + 384], slot[:, b:b + 384],
                                master[:, 1920 - MTRIM:2304 - MTRIM], ALU.add,
                            )
                        h = 2 * dqt + par
                        nc.scalar.activation(
                            pt[:, 384 * par:384 * par + 384], slot[:, b:b + 384],
                            AF.Exp, scale=float(s_heads[h]),
                        )
                    pts.append(pt)
                    if pump is not None and cl == 1:
                        pump(1)
                if pump is not None:
                    pump(1)
                # PV by j-tile; users(jt) = chunks c with jt in {c-1,c,c+1}
                first = {0: True, 1: True}
                qjts = [(4 * qh - 1) % 16] + list(range(4 * qh, 4 * qh + 5))
                for n_, jt in enumerate(qjts):
                    for par in range(2):
                        h = 2 * dqt + par
                        for cl in range(4):
                            c = 4 * qh + cl
                            # slice k of chunk c holds jt = (c+1-k) mod 16
                            dk = None
                            for k in range(3):
                                if (c + 1 - k) % 16 == jt:
                                    dk = k
                            if dk is None:
                                continue
                            last = (n_ == len(qjts) - 1 and cl == 3)
                            nc.tensor.matmul(
                                po[par][:, 128 * cl:128 * cl + 128],
                                vsb_slice(jt, h),
                                pts[cl][:, 384 * par + 128 * dk:384 * par + 128 * dk + 128],
                                start=first[par], stop=last,
                            )
                            first[par] = False
                # stash O^T + softmax sums
                for par in range(2):
                    h = 2 * dqt + par
                    lt = ltp.tile([1, 512], F32, tag="lt")
                    nc.scalar.copy(lt[:], po[par][64:65, :])
                    nc.sync.dma_start(lrows[h:h + 1, :], lt[:])
                    if par == 0:
                        nc.scalar.copy(
                            OT[0:64, dqt, ts(qh, 512)], po[par][0:64, :]
                        )
                    else:
                        tmp = epool.tile([64, 512], BF16, tag="otmp")
                        nc.scalar.copy(tmp[:], po[par][0:64, :])
                        nc.sync.dma_start(OT[64:128, dqt, ts(qh, 512)], tmp[:])

            # ---------- attention: pair 7 (h14 banded-512, h15 full) ----
            def emit_pair_big(qh, lrows, pump=None):
                dqt = 7
                blocks = {0: _band_blocks(Ts[14], qh), 1: _band_blocks(CTX, qh)}
                po = {}
                for par in range(2):
                    po[par] = pso.tile([65, 512], F32, tag="po", name=f"po{par}")
                first = {0: True, 1: True}

                def groups(par):
                    bl = blocks[par]
                    return [bl[i:i + 2] for i in range(0, len(bl), 2)]

                g14, g15 = groups(0), groups(1)
                ngr = max(len(g14), len(g15))
                for gi in range(ngr):
                    if pump is not None and gi % 2 == 1:
                        pump(1)
                    for par, grp_list in ((0, g14), (1, g15)):
                        if gi >= len(grp_list):
                            continue
                        grp = grp_list[gi]
                        h = 14 + par
                        b = 64 * par
                        slot = psA.tile([128, 1024], F32, tag="psA")
                        for g2, jt in enumerate(grp):
                            col = kt_col(dqt, jt)
                            nc.tensor.matmul(
                                slot[:, 512 * g2:512 * g2 + 512],
                                KT[b:b + 64, col:col + 128],
                                QT[b:b + 64, dqt, ts(qh, 512)],
                                start=True, stop=True,
                            )
                        for g2, jt in enumerate(grp):
                            off = 2048 + 512 * qh - 128 * jt
                            mt, mo = msrc(jt, off)
                            nc.vector.tensor_tensor(
                                slot[:, 512 * g2:512 * g2 + 512],
                                slot[:, 512 * g2:512 * g2 + 512],
                                mt[:, mo:mo + 512], ALU.add,
                            )
                        pt = ptp.tile([128, 1024], BF16, tag="pt")
                        nc.scalar.activation(
                            pt[:, 0:512 * len(grp)], slot[:, 0:512 * len(grp)],
                            AF.Exp, scale=float(s_heads[h]),
                        )
                        for g2, jt in enumerate(grp):
                            last = (gi == len(grp_list) - 1 and
                                    g2 == len(grp) - 1)
                            nc.tensor.matmul(
                                po[par][:], vsb_slice(jt, h),
                                pt[:, ts(g2, 512)],
                                start=first[par], stop=last,
                            )
                            first[par] = False
                for par in range(2):
                    h = 14 + par
                    lt = ltp.tile([1, 512], F32, tag="lt")
                    nc.scalar.copy(lt[:], po[par][64:65, :])
                    nc.sync.dma_start(lrows[h:h + 1, :], lt[:])
                    if par == 0:
                        nc.scalar.copy(
                            OT[0:64, dqt, ts(qh, 512)], po[par][0:64, :]
                        )
                    else:
                        tmp = epool.tile([64, 512], BF16, tag="otmp")
                        nc.scalar.copy(tmp[:], po[par][0:64, :])
                        nc.sync.dma_start(OT[64:128, dqt, ts(qh, 512)], tmp[:])

            # ---------- softmax normalization for one q-half ----------
            def emit_norm(qh, lrows):
                linv = lp.tile([16, 512], F32R, tag="linv")
                with nc.allow_low_precision(reason="1/l broadcast via f32r matmul"):
                    nc.vector.reciprocal(linv[:], lrows[:])
                for j in range(8):
                    pb = psC.tile([128, 512], F32, tag="psC")
                    nc.tensor.matmul(
                        pb[:], oh16[:, ts(j, 128)], linv[:],
                        start=True, stop=True,
                    )
                    nc.vector.tensor_tensor(
                        OT[:, j, ts(qh, 512)], OT[:, j, ts(qh, 512)],
                        pb[:], ALU.mult,
                    )

            # ---------- output projection for one q-half ----------
            def emit_wot_dma(ec):
                # wv_sb is dead after the last V unit; reuse it as the
                # out-projection weight buffer (both 512-col halves resident)
                nc.sync.dma_start(
                    wv_sb[:, :, ts(ec, 512)],
                    wo_d[:, :, ts(ec, 512)].rearrange("h p e -> p h e"),
                )

            def emit_outproj_unit(ec, itl):
                ps = psC.tile([128, 512], F32, tag="psC")
                for hdt in range(8):
                    nc.tensor.matmul(
                        ps[:], OT[:, hdt, ts(itl, 128)],
                        wv_sb[:, hdt, ts(ec, 512)],
                        start=(hdt == 0), stop=(hdt == 7),
                    )
                ot = op.tile([128, 512], F32, tag="ot")
                nc.vector.tensor_copy(ot[:], ps[:])
                nc.sync.dma_start(out_d[ts(itl, 128), ts(ec, 512)], ot[:])

            # =================== emission schedule ===================
            # Wave A: all LN/transposes (DVE-heavy, overlaps dense PE
            # projections), K for j-tiles 0..7 + 15, Q for q-half 0,
            # V j-tiles 0..4 + 15.
            emit_chunk(0)
            for dqt in range(4):
                emit_q_half(dqt, 0)
            emit_chunk(1)
            for dqt in range(4, 8):
                emit_q_half(dqt, 0)
            emit_chunk(3)
            emit_chunk(2)
            emit_k_waveA()
            for jt in range(5):
                emit_v_full(jt, psA)
            emit_k_band(15, range(7))
            emit_v_full(15, psA)

            # Filler closures drained inside the attention pair emission.
            fillA_dep = []
            fillA_dep.append(lambda: emit_k_band(8, range(0, 4)))
            fillA_dep.append(lambda: emit_k_band(8, range(4, 7)))
            fillA_dep.append(lambda: emit_k_dqt7_hi())
            fillA_dep.append(lambda: emit_v_full(8, psC))
            fillA_dep.append(lambda: emit_v_b((9, 10, 11)))
            fillA_dep.append(lambda: emit_v_b((12, 13, 14)))
            for jt in (5, 6, 7):
                fillA_dep.append(lambda j=jt: emit_v_full(j, psC))
            fillA_free = []
            for dqt in range(8):
                fillA_free.append(lambda d=dqt: emit_q_half(d, 1))
            fillA = fillA_dep  # pairs drain deps first


            def mkpump(queue):
                def pump(n):
                    for _ in range(n):
                        if queue:
                            queue.pop(0)()
                return pump

            def pumpA(n):
                for _ in range(n):
                    if fillA_dep:
                        fillA_dep.pop(0)()
                    elif fillA_free:
                        fillA_free.pop(0)()

            lrows0 = lp.tile([16, 512], F32, tag="lrows")
            for dqt in range(7):
                emit_pair_small(dqt, 0, lrows0, pumpA)
            # pair_big(0) reads every V/K tile: those must all be emitted
            pumpA(len(fillA_dep))
            emit_pair_big(0, lrows0, pumpA)
            pumpA(len(fillA_free))  # remaining Q-qh1 before qh1 pairs
            emit_norm(0, lrows0)

            # qh1 attention with qh0 out-projection as filler
            emit_wot_dma(0)
            emit_wot_dma(1)
            fillB = []
            for itl in range(4):
                for ec in range(2):
                    fillB.append(lambda e=ec, i=itl: emit_outproj_unit(e, i))
            pumpB = mkpump(fillB)
            calls = [0]

            def pumpB_r(n):
                calls[0] += 1
                if calls[0] % 3 == 0:
                    pumpB(n)

            lrows1 = lp.tile([16, 512], F32, tag="lrows")
            for dqt in range(7):
                emit_pair_small(dqt, 1, lrows1, pumpB_r)
            emit_pair_big(1, lrows1, pumpB_r)
            emit_norm(1, lrows1)
            pumpB(len(fillB))  # leftovers cover norm1's serial chain
            # qh1 out-projection (tail; weights already resident)
            for itl in range(4, 8):
                for ec in range(2):
                    emit_outproj_unit(ec, itl)

    nc.compile()
    return nc


def _prep(x, ln_w, ln_b, Wq, Wk, Wv, Wo, M):
    x = np.asarray(x, np.float32)
    ln_w = np.asarray(ln_w, np.float32)
    ln_b = np.asarray(ln_b, np.float32)
    Wq = np.asarray(Wq, np.float32)
    Wk = np.asarray(Wk, np.float32)
    Wv = np.asarray(Wv, np.float32)
    Wo = np.asarray(Wo, np.float32)
    M = np.asarray(M, np.float32)
    assert not np.any(ln_b), "kernel assumes ln_b == 0"

    s_heads = (-M[:, 0, 1]).astype(np.float64)  # M[h,0,1] = -s_h
    Ts = [min(CTX, int(np.ceil(19.0 / s))) for s in s_heads]
    assert all(t <= 127 for t in Ts[:14]), "P3 pattern needs T<=127 for h0..13"

    wq_eff = ln_w[:, None] * Wq
    for h in range(NH):
        wq_eff[:, h * DH:(h + 1) * DH] /= 8.0 * s_heads[h]
    wk_eff = ln_w[:, None] * Wk
    wv_eff = ln_w[:, None] * Wv

    def wq_layout(w):  # [1024,1024] -> [dqt, p, ko, m]
        return np.ascontiguousarray(
            w.reshape(8, 128, 8, 128).transpose(2, 1, 0, 3)
        ).astype(ml_dtypes.bfloat16)

    wq_a = wq_layout(wq_eff)
    wk_a = wq_layout(wk_eff)
    wv_a = np.ascontiguousarray(wv_eff.reshape(8, 128, DIM)).astype(
        ml_dtypes.bfloat16
    )
    wo_a = np.ascontiguousarray(Wo.reshape(8, 128, DIM)).astype(ml_dtypes.bfloat16)

    ident = np.eye(128, dtype=np.float32).astype(ml_dtypes.bfloat16)
    oh = np.zeros((16, 2048), np.float32)
    for j in range(8):
        oh[2 * j, 128 * j:128 * j + 64] = 1.0
        oh[2 * j + 1, 128 * j + 64:128 * (j + 1)] = 1.0

    # master[pj, u]: r = u - pj - 2048 (= i_local - j_local)
    u = np.arange(3072, dtype=np.float64)[None, :]
    pj = np.arange(128, dtype=np.float64)[:, None]
    r = u - pj - 2048.0

    def _bf(a):
        return np.ascontiguousarray(
            np.maximum(a, -20000.0).astype(np.float32)
        ).astype(ml_dtypes.bfloat16)

    m0 = _bf(-np.abs(r[:, MTRIM:]))
    masters1 = [_bf(-np.abs(r[:, :2048])), _bf(-np.abs(r[:, :2048] + 2048.0))]

    in_maps = []
    for c in range(8):
        b, t = c // 2, c % 2
        xr = np.ascontiguousarray(np.roll(x[b], -QTOK * t, axis=0)).astype(
            ml_dtypes.bfloat16
        )
        in_maps.append({
            "x": xr, "wq": wq_a, "wk": wk_a, "wv": wv_a, "wo": wo_a,
            "master": m0, "master1": masters1[t], "ident": ident, "oh16": oh,
        })
    return s_heads, Ts, in_maps


def kernel(**inputs):
    global LAST_EXEC_NS
    s_heads, Ts, in_maps = _prep(**inputs)
    nc = _build_graph(s_heads, Ts)
    trace = os.environ.get("KERNEL_TRACE") == "1"
    res = run_bass_kernel_spmd(
        nc, in_maps, core_ids=list(range(8)), trace=trace
    )
    LAST_EXEC_NS = res.exec_time_ns
    out = np.empty((4, CTX, DIM), np.float32)
    for c in range(8):
        b, t = c // 2, c % 2
        out[b, QTOK * t:QTOK * (t + 1), :] = res.results[c]["out"]
    return out


# revision 23
# speedup vs baseline: 1.0069x; 1.0069x over previous
"""Distributed Bass kernel: LN + multi-head ALiBi attention + out-proj (v3).

Sharding: 8 cores = (batch b in 0..3) x (query-token half t in 0..1).
Each core computes the full pipeline for its 1024 query tokens (all 16
heads); K/V are computed for the 2048-token sequence (duplicated across
the 2 cores of a batch), restricted to the union of the heads' ALiBi
bands.  No collectives.

SPMD trick: every core runs the SAME graph.  Core (b, t) receives x[b]
rolled by -1024*t along tokens, so its query tokens always sit at local
rows 0..1023.  Distance tables (master/master1) are per-core DATA.

v3 structure (vs the W=512 baseline):
  * ALiBi banding: head h attends within |i-j| <= T_h = ceil(19/s_h).
    Heads 0..13 (T<=121) use diagonal-aligned [128q x 3*128j] blocks:
    query chunk c attends j-tiles {c-1, c, c+1} (mod 16).  This halves
    score/bias/exp/PV volume vs 512-wide blocks, and the bias slice for
    a chunk is ONE contiguous master window (identical for all chunks).
  * K/V projections skip (head-group, j-tile) units outside the bands;
    projection loops reuse each loaded weight for 2+ matmuls.
  * Head pairs (2k, 2k+1) occupy PE row groups [0:64]/[64:128]; their
    k=64 score matmuls run concurrently (row tiling) and LDWEIGHTS of
    one parity overlaps the other parity's matmul.
  * Score bias is added in PSUM in place (DVE) and exp reads PSUM.
  * Softmax denominators ride the PV matmul as a ones-column (row 64),
    are collected into lrows[16,512] per q-half, inverted once with
    DVE reciprocal, and broadcast back with a one-hot matmul (no Ln/
    exp(-x) chain, no ACT table thrashing).
  * Emission interleaves late projection work into the attention stream
    so the PE never idles long enough to re-throttle (HAM).
"""

import os
import sys

sys.path.insert(0, "/opt/trn_rl_repo")

import numpy as np
import ml_dtypes

import concourse.bass as bass
import concourse.mybir as mybir
import concourse.tile as tile
from concourse import bacc
from concourse.bass import ts
from concourse.bass_utils import run_bass_kernel_spmd

BF16 = mybir.dt.bfloat16
F32 = mybir.dt.float32
F32R = mybir.dt.float32r

CTX = 2048
DIM = 1024
NH = 16
DH = 64
QTOK = 1024
EPS = 1e-5
MTRIM = 1152  # master table column trim (cols < 1152 never read)
MW = 3072 - MTRIM

LAST_EXEC_NS = None


def _slotX(jt):
    """xnT/KT/Vsb packed slot order: [0..8, 15, 9..14]."""
    if jt <= 8:
        return jt
    if jt == 15:
        return 9
    return jt + 1


def _band_blocks(T, qc):
    """W=512 j-tile list for query chunk qc (g0 = qc*512), half-width T."""
    g0 = qc * 512
    lo = max(0, g0 - T) // 128
    hi = (min(CTX, g0 + 512 + T) + 127) // 128
    jts = set(range(lo, hi))
    if qc == 0 and T < CTX:
        jts |= set(range((CTX - T) // 128, CTX // 128))
    return sorted(jts)


def _build_graph(s_heads, Ts):
    nc = bacc.Bacc("TRN2", target_bir_lowering=False, debug=False)

    x_d = nc.dram_tensor("x", [CTX, DIM], BF16, kind="ExternalInput").ap()
    wq_d = nc.dram_tensor("wq", [8, 128, 8, 128], BF16, kind="ExternalInput").ap()
    wk_d = nc.dram_tensor("wk", [8, 128, 8, 128], BF16, kind="ExternalInput").ap()
    wv_d = nc.dram_tensor("wv", [8, 128, DIM], BF16, kind="ExternalInput").ap()
    wo_d = nc.dram_tensor("wo", [8, 128, DIM], BF16, kind="ExternalInput").ap()
    mst_d = nc.dram_tensor("master", [128, MW], BF16, kind="ExternalInput").ap()
    mst1_d = nc.dram_tensor("master1", [128, 2048], BF16, kind="ExternalInput").ap()
    idn_d = nc.dram_tensor("ident", [128, 128], BF16, kind="ExternalInput").ap()
    oh_d = nc.dram_tensor("oh16", [16, 2048], F32R, kind="ExternalInput").ap()
    out_d = nc.dram_tensor("out", [QTOK, DIM], F32, kind="ExternalOutput").ap()

    AF = mybir.ActivationFunctionType
    ALU = mybir.AluOpType

    KTW = 7 * 1280 + 2048  # banded KT: 10 slots for dqt<7, 16 for dqt7

    def kt_col(dqt, jt):
        return min(dqt, 7) * 1280 + 128 * _slotX(jt)

    with tile.TileContext(nc) as tc:
        with (
            tc.tile_pool(name="persist", bufs=1) as pp,
            tc.tile_pool(name="xio", bufs=5) as xp,
            tc.tile_pool(name="xnp", bufs=4) as xnp,
            tc.tile_pool(name="wstream", bufs=3) as wp,
            tc.tile_pool(name="ptp", bufs=4) as ptp,
            tc.tile_pool(name="small", bufs=4) as sp,
            tc.tile_pool(name="lpool", bufs=2) as lp,
            tc.tile_pool(name="ltp", bufs=2) as ltp,
            tc.tile_pool(name="epool", bufs=2) as epool,
            tc.tile_pool(name="wopool", bufs=1) as wop,
            tc.tile_pool(name="opool", bufs=2) as op,
            tc.tile_pool(name="psA", bufs=2, space="PSUM") as psA,
            tc.tile_pool(name="psC", bufs=2, space="PSUM") as psC,
            tc.tile_pool(name="pso", bufs=2, space="PSUM") as pso,
        ):
            # ---- persistent SBUF ----
            master = pp.tile([128, MW], BF16, tag="master")
            master1 = pp.tile([128, 2048], BF16, tag="master1")
            ident = pp.tile([128, 128], BF16, tag="ident")
            oh16 = pp.tile([16, 2048], F32R, tag="oh16")
            xnT = pp.tile([128, 8, CTX], BF16, tag="xnT")
            KT = pp.tile([128, KTW], BF16, tag="KT")
            QT = pp.tile([128, 8, QTOK], BF16, tag="QT")
            VsbA = pp.tile([128, 10, NH, 65], BF16, tag="VsbA")
            VsbB = pp.tile([128, 6, 2, 65], BF16, tag="VsbB")
            OT = pp.tile([128, 8, QTOK], BF16, tag="OT")
            wv_sb = pp.tile([128, 8, DIM], BF16, tag="wv")
            eps_sb = pp.tile([128, 1], F32, tag="eps")

            nc.scalar.dma_start(master[:], mst_d[:])
            nc.scalar.dma_start(master1[:], mst1_d[:])
            nc.scalar.dma_start(ident[:], idn_d[:])
            nc.scalar.dma_start(oh16[:], oh_d[:])
            nc.scalar.dma_start(wv_sb[:], wv_d.rearrange("k p d -> p k d"))
            nc.any.memset(eps_sb[:], EPS)
            nc.any.memset(VsbA[:, :, :, 64:65], 1.0)
            nc.any.memset(VsbB[:, :, :, 64:65], 1.0)

            def vsb_slice(jt, h):
                if jt in (9, 10, 11, 12, 13, 14):
                    assert h >= 14
                    return VsbB[:, jt - 9, h - 14, 0:65]
                return VsbA[:, _slotX(jt), h, 0:65]

            def msrc(jt, off):
                """(tile, col) for a W=512 bias slice at original col off."""
                if jt >= 8:
                    return master1, off
                return master, off - MTRIM

            # ---------- LN + transpose for one 512-token chunk ----------
            def emit_chunk(ch):
                mv4 = sp.tile([128, 4, 2], F32, tag="mv4")
                xts = []
                for i, tt in enumerate(range(4 * ch, 4 * ch + 4)):
                    xt = xp.tile([128, DIM], BF16, tag="xt")
                    nc.sync.dma_start(xt[:], x_d[ts(tt, 128), :])
                    xts.append(xt)
                    st6 = sp.tile([128, 2, 6], F32, tag="st6")
                    nc.vector.bn_stats(st6[:, 0, :], xt[:, 0:512])
                    nc.vector.bn_stats(st6[:, 1, :], xt[:, 512:1024])
                    nc.vector.bn_aggr(mv4[:, i, :], st6[:])
                lv4 = sp.tile([128, 4], F32, tag="lv4")
                nc.scalar.activation(lv4[:], mv4[:, :, 1], AF.Ln, bias=eps_sb[:])
                rs4 = sp.tile([128, 4], F32, tag="rs4")
                nc.scalar.activation(rs4[:], lv4[:], AF.Exp, scale=-0.5)
                xns = []
                for i in range(4):
                    xn = xnp.tile([128, DIM], BF16, tag="xn")
                    nc.vector.tensor_scalar(
                        xn[:], xts[i][:], mv4[:, i, 0:1], rs4[:, i:i + 1],
                        ALU.subtract, ALU.mult,
                    )
                    xns.append(xn)
                # transpose on PE -> xnT packed slots
                jts = list(range(4 * ch, 4 * ch + 4))
                for ko in range(8):
                    tp = psC.tile([128, 512], F32, tag="psC")
                    tpv = tp[:].bitcast(BF16)
                    for i in range(4):
                        nc.tensor.transpose(
                            tpv[:, ts(i, 128)], xns[i][:, ts(ko, 128)], ident[:]
                        )
                    # evacuate to packed slots (contiguous runs)
                    runs = []
                    start = 0
                    while start < 4:
                        end = start
                        while (end + 1 < 4 and
                               _slotX(jts[end + 1]) == _slotX(jts[end]) + 1):
                            end += 1
                        runs.append((start, end))
                        start = end + 1
                    for (a, b) in runs:
                        w = 128 * (b - a + 1)
                        c0 = 128 * _slotX(jts[a])
                        nc.vector.tensor_copy(
                            xnT[:, ko, c0:c0 + w], tpv[:, 128 * a:128 * a + w]
                        )

            # ---------- projection helpers ----------
            def emit_k_waveA():
                """K projection over chunks 0,1 (xnT cols 0:1024)."""
                for dqt in range(8):
                    wt = wp.tile([128, 8, 128], BF16, tag="wt")
                    nc.scalar.dma_start(wt[:], wk_d[dqt])
                    ps = psA.tile([128, 1024], F32, tag="psA")
                    for ko in range(8):
                        nc.tensor.matmul(
                            ps[:, 0:512], wt[:, ko, :], xnT[:, ko, 0:512],
                            start=(ko == 0), stop=(ko == 7),
                        )
                        nc.tensor.matmul(
                            ps[:, 512:1024], wt[:, ko, :], xnT[:, ko, 512:1024],
                            start=(ko == 0), stop=(ko == 7),
                        )
                    c0 = kt_col(dqt, 0)
                    nc.vector.tensor_copy(KT[:, c0:c0 + 1024], ps[:])

            def emit_q_half(dqt, qh):
                """Q projection for one dqt, one 512-token query half."""
                wt = wp.tile([128, 8, 128], BF16, tag="wt")
                nc.scalar.dma_start(wt[:], wq_d[dqt])
                ps = psC.tile([128, 512], F32, tag="psC")
                for ko in range(8):
                    nc.tensor.matmul(
                        ps[:], wt[:, ko, :], xnT[:, ko, ts(qh, 512)],
                        start=(ko == 0), stop=(ko == 7),
                    )
                nc.vector.tensor_copy(QT[:, dqt, ts(qh, 512)], ps[:])

            def emit_k_band(tile_jt, dqts):
                """K^T band tile (jt 8 or 15) for a subset of dqt<=6."""
                sx = 128 * _slotX(tile_jt)
                for dqt in dqts:
                    wt = wp.tile([128, 8, 128], BF16, tag="wt")
                    nc.scalar.dma_start(wt[:], wk_d[dqt])
                    ps = psC.tile([128, 512], F32, tag="psC")
                    for ko in range(8):
                        nc.tensor.matmul(
                            ps[:, 0:128], wt[:, ko, :],
                            xnT[:, ko, sx:sx + 128],
                            start=(ko == 0), stop=(ko == 7),
                        )
                    c0 = kt_col(dqt, tile_jt)
                    nc.vector.tensor_copy(KT[:, c0:c0 + 128], ps[:, 0:128])

            def emit_k_dqt7_hi():
                """dqt7 K^T for xnT slots 8..15 (cols 1024:2048), psC pair."""
                wt = wp.tile([128, 8, 128], BF16, tag="wt")
                nc.scalar.dma_start(wt[:], wk_d[7])
                for half in range(2):
                    ps = psC.tile([128, 512], F32, tag="psC")
                    for ko in range(8):
                        nc.tensor.matmul(
                            ps[:], wt[:, ko, :],
                            xnT[:, ko, 1024 + 512 * half:1536 + 512 * half],
                            start=(ko == 0), stop=(ko == 7),
                        )
                    c0 = 7 * 1280 + 1024 + 512 * half
                    nc.vector.tensor_copy(KT[:, c0:c0 + 512], ps[:])

            def emit_v_full(jt, pool):
                """V proj for all 16 heads at j-tile jt (slot in VsbA)."""
                sx = 128 * _slotX(jt)
                if pool is psA:
                    ps = psA.tile([128, 1024], F32, tag="psA")
                    for ko in range(8):
                        nc.tensor.matmul(
                            ps[:, 0:512], xnT[:, ko, sx:sx + 128],
                            wv_sb[:, ko, 0:512],
                            start=(ko == 0), stop=(ko == 7),
                        )
                        nc.tensor.matmul(
                            ps[:, 512:1024], xnT[:, ko, sx:sx + 128],
                            wv_sb[:, ko, 512:1024],
                            start=(ko == 0), stop=(ko == 7),
                        )
                    nc.vector.tensor_copy(
                        VsbA[:, _slotX(jt), :, 0:64],
                        ps[:].rearrange("p (h d) -> p h d", d=64),
                    )
                else:
                    for half in range(2):
                        ps = psC.tile([128, 512], F32, tag="psC")
                        for ko in range(8):
                            nc.tensor.matmul(
                                ps[:], xnT[:, ko, sx:sx + 128],
                                wv_sb[:, ko, ts(half, 512)],
                                start=(ko == 0), stop=(ko == 7),
                            )
                        nc.vector.tensor_copy(
                            VsbA[:, _slotX(jt), 8 * half:8 * half + 8, 0:64],
                            ps[:].rearrange("p (h d) -> p h d", d=64),
                        )

            def emit_v_b(jts):
                """V proj for heads 14,15 only (VsbB), jts subset of 9..14."""
                for jt in jts:
                    sx = 128 * _slotX(jt)
                    ps = psC.tile([128, 512], F32, tag="psC")
                    for ko in range(8):
                        nc.tensor.matmul(
                            ps[:, 0:128], xnT[:, ko, sx:sx + 128],
                            wv_sb[:, ko, 896:1024],
                            start=(ko == 0), stop=(ko == 7),
                        )
                    nc.vector.tensor_copy(
                        VsbB[:, jt - 9, :, 0:64],
                        ps[:, 0:128].rearrange("p (h d) -> p h d", d=64),
                    )

            # ---------- attention: banded pair (heads 2k,2k+1), W=128 ----
            def emit_pair_small(dqt, qh, lrows, pump=None):
                po = {}
                for par in range(2):
                    po[par] = pso.tile([65, 512], F32, tag="po", name=f"po{par}")
                pts = []
                for cl in range(4):
                    c = 4 * qh + cl
                    slot = psA.tile([128, 1024], F32, tag="psA")
                    for k in range(3):
                        jt = (c + 1 - k) % 16
                        col = kt_col(dqt, jt)
                        for par in range(2):
                            b = 64 * par
                            nc.tensor.matmul(
                                slot[:, 512 * par + 128 * k:512 * par + 128 * k + 128],
                                KT[b:b + 64, col:col + 128],
                                QT[b:b + 64, dqt, 128 * c:128 * c + 128],
                                start=(k == 0), stop=(k == 2),
                            )
                    pt = ptp.tile([128, 1024], BF16, tag="pt")
                    for par in range(2):
                        b = 512 * par
                        if c == 0:
                            # k=0,1 from master, k=2 (wrap jt 15) from master1
                            nc.vector.tensor_tensor(
                                slot[:, b:b + 256], slot[:, b:b + 256],
                                master[:, 1920 - MTRIM:2176 - MTRIM], ALU.add,
                            )
                            nc.vector.tensor_tensor(
                                slot[:, b + 256:b + 384], slot[:, b + 256:b + 384],
                                master1[:, 128:256], ALU.add,
                            )
                        elif c == 7:
                            # k=0 (jt 8 crosses the roll-wrap plane) -> master1
                            nc.vector.tensor_tensor(
                                slot[:, b:b + 128], slot[:, b:b + 128],
                                master1[:, 1920:2048], ALU.add,
                            )
                            nc.vector.tensor_tensor(
                                slot[:, b + 128:b + 384], slot[:, b + 128:b + 384],
                                master[:, 2048 - MTRIM:2304 - MTRIM], ALU.add,
                            )
                        else:
                            nc.vector.tensor_tensor(
                                slot[:, b:b + 384], slot[:, b:b + 384],
                                master[:, 1920 - MTRIM:2304 - MTRIM], ALU.add,
                            )
                        h = 2 * dqt + par
                        nc.scalar.activation(
                            pt[:, 384 * par:384 * par + 384], slot[:, b:b + 384],
                            AF.Exp, scale=float(s_heads[h]),
                        )
                    pts.append(pt)
                    if pump is not None and cl == 1:
                        pump(1)
                if pump is not None:
                    pump(1)
                # PV by j-tile; users(jt) = chunks c with jt in {c-1,c,c+1}
                first = {0: True, 1: True}
                qjts = [(4 * qh - 1) % 16] + list(range(4 * qh, 4 * qh + 5))
                for n_, jt in enumerate(qjts):
                    for par in range(2):
                        h = 2 * dqt + par
                        for cl in range(4):
                            c = 4 * qh + cl
                            # slice k of chunk c holds jt = (c+1-k) mod 16
                            dk = None
                            for k in range(3):
                                if (c + 1 - k) % 16 == jt:
                                    dk = k
                            if dk is None:
                                continue
                            last = (n_ == len(qjts) - 1 and cl == 3)
                            nc.tensor.matmul(
                                po[par][:, 128 * cl:128 * cl + 128],
                                vsb_slice(jt, h),
                                pts[cl][:, 384 * par + 128 * dk:384 * par + 128 * dk + 128],
                                start=first[par], stop=last,
                            )
                            first[par] = False
                # stash O^T + softmax sums
                for par in range(2):
                    h = 2 * dqt + par
                    lt = ltp.tile([1, 512], F32, tag="lt")
                    nc.scalar.copy(lt[:], po[par][64:65, :])
                    nc.sync.dma_start(lrows[h:h + 1, :], lt[:])
                    if par == 0:
                        nc.vector.tensor_copy(
                            OT[0:64, dqt, ts(qh, 512)], po[par][0:64, :]
                        )
                    else:
                        tmp = epool.tile([64, 512], BF16, tag="otmp")
                        nc.vector.tensor_copy(tmp[:], po[par][0:64, :])
                        nc.sync.dma_start(OT[64:128, dqt, ts(qh, 512)], tmp[:])

            # ---------- attention: pair 7 (h14 banded-512, h15 full) ----
            def emit_pair_big(qh, lrows, pump=None):
                dqt = 7
                blocks = {0: _band_blocks(Ts[14], qh), 1: _band_blocks(CTX, qh)}
                po = {}
                for par in range(2):
                    po[par] = pso.tile([65, 512], F32, tag="po", name=f"po{par}")
                first = {0: True, 1: True}

                def groups(par):
                    bl = blocks[par]
                    return [bl[i:i + 2] for i in range(0, len(bl), 2)]

                g14, g15 = groups(0), groups(1)
                ngr = max(len(g14), len(g15))
                for gi in range(ngr):
                    if pump is not None:
                        pump(1)
                    for par, grp_list in ((0, g14), (1, g15)):
                        if gi >= len(grp_list):
                            continue
                        grp = grp_list[gi]
                        h = 14 + par
                        b = 64 * par
                        slot = psA.tile([128, 1024], F32, tag="psA")
                        for g2, jt in enumerate(grp):
                            col = kt_col(dqt, jt)
                            nc.tensor.matmul(
                                slot[:, 512 * g2:512 * g2 + 512],
                                KT[b:b + 64, col:col + 128],
                                QT[b:b + 64, dqt, ts(qh, 512)],
                                start=True, stop=True,
                            )
                        for g2, jt in enumerate(grp):
                            off = 2048 + 512 * qh - 128 * jt
                            mt, mo = msrc(jt, off)
                            nc.vector.tensor_tensor(
                                slot[:, 512 * g2:512 * g2 + 512],
                                slot[:, 512 * g2:512 * g2 + 512],
                                mt[:, mo:mo + 512], ALU.add,
                            )
                        pt = ptp.tile([128, 1024], BF16, tag="pt")
                        nc.scalar.activation(
                            pt[:, 0:512 * len(grp)], slot[:, 0:512 * len(grp)],
                            AF.Exp, scale=float(s_heads[h]),
                        )
                        for g2, jt in enumerate(grp):
                            last = (gi == len(grp_list) - 1 and
                                    g2 == len(grp) - 1)
                            nc.tensor.matmul(
                                po[par][:], vsb_slice(jt, h),
                                pt[:, ts(g2, 512)],
                                start=first[par], stop=last,
                            )
                            first[par] = False
                for par in range(2):
                    h = 14 + par
                    lt = ltp.tile([1, 512], F32, tag="lt")
                    nc.scalar.copy(lt[:], po[par][64:65, :])
                    nc.sync.dma_start(lrows[h:h + 1, :], lt[:])
                    if par == 0:
                        nc.vector.tensor_copy(
                            OT[0:64, dqt, ts(qh, 512)], po[par][0:64, :]
                        )
                    else:
                        tmp = epool.tile([64, 512], BF16, tag="otmp")
                        nc.vector.tensor_copy(tmp[:], po[par][0:64, :])
                        nc.sync.dma_start(OT[64:128, dqt, ts(qh, 512)], tmp[:])

            # ---------- softmax normalization for one q-half ----------
            def emit_norm(qh, lrows):
                linv = lp.tile([16, 512], F32R, tag="linv")
                with nc.allow_low_precision(reason="1/l broadcast via f32r matmul"):
                    nc.vector.reciprocal(linv[:], lrows[:])
                for j in range(8):
                    pb = psC.tile([128, 512], F32, tag="psC")
                    nc.tensor.matmul(
                        pb[:], oh16[:, ts(j, 128)], linv[:],
                        start=True, stop=True,
                    )
                    nc.vector.tensor_tensor(
                        OT[:, j, ts(qh, 512)], OT[:, j, ts(qh, 512)],
                        pb[:], ALU.mult,
                    )

            # ---------- output projection for one q-half ----------
            def emit_wot_dma(ec):
                # wv_sb is dead after the last V unit; reuse it as the
                # out-projection weight buffer (both 512-col halves resident)
                nc.sync.dma_start(
                    wv_sb[:, :, ts(ec, 512)],
                    wo_d[:, :, ts(ec, 512)].rearrange("h p e -> p h e"),
                )

            def emit_outproj_unit(ec, itl):
                ps = psC.tile([128, 512], F32, tag="psC")
                for hdt in range(8):
                    nc.tensor.matmul(
                        ps[:], OT[:, hdt, ts(itl, 128)],
                        wv_sb[:, hdt, ts(ec, 512)],
                        start=(hdt == 0), stop=(hdt == 7),
                    )
                ot = op.tile([128, 512], F32, tag="ot")
                nc.vector.tensor_copy(ot[:], ps[:])
                nc.sync.dma_start(out_d[ts(itl, 128), ts(ec, 512)], ot[:])

            # =================== emission schedule ===================
            # Wave A: all LN/transposes (DVE-heavy, overlaps dense PE
            # projections), K for j-tiles 0..7 + 15, Q for q-half 0,
            # V j-tiles 0..4 + 15.
            emit_chunk(0)
            for dqt in range(4):
                emit_q_half(dqt, 0)
            emit_chunk(1)
            for dqt in range(4, 8):
                emit_q_half(dqt, 0)
            emit_chunk(3)
            emit_chunk(2)
            emit_k_waveA()
            for jt in range(5):
                emit_v_full(jt, psA)
            emit_k_band(15, range(7))
            emit_v_full(15, psA)

            # Filler closures drained inside the attention pair emission.
            fillA_dep = []
            fillA_dep.append(lambda: emit_k_band(8, range(0, 4)))
            fillA_dep.append(lambda: emit_k_band(8, range(4, 7)))
            fillA_dep.append(lambda: emit_k_dqt7_hi())
            fillA_dep.append(lambda: emit_v_full(8, psC))
            fillA_dep.append(lambda: emit_v_b((9, 10, 11)))
            fillA_dep.append(lambda: emit_v_b((12, 13, 14)))
            for jt in (5, 6, 7):
                fillA_dep.append(lambda j=jt: emit_v_full(j, psC))
            fillA_free = []
            for dqt in range(8):
                fillA_free.append(lambda d=dqt: emit_q_half(d, 1))
            fillA = fillA_dep  # pairs drain deps first


            def mkpump(queue):
                def pump(n):
                    for _ in range(n):
                        if queue:
                            queue.pop(0)()
                return pump

            def pumpA(n):
                for _ in range(n):
                    if fillA_dep:
                        fillA_dep.pop(0)()
                    elif fillA_free:
                        fillA_free.pop(0)()

            lrows0 = lp.tile([16, 512], F32, tag="lrows")
            for dqt in range(7):
                emit_pair_small(dqt, 0, lrows0, pumpA)
            # pair_big(0) reads every V/K tile: those must all be emitted
            pumpA(len(fillA_dep))
            emit_pair_big(0, lrows0, pumpA)
            pumpA(len(fillA_free))  # remaining Q-qh1 before qh1 pairs
            emit_norm(0, lrows0)

            # qh1 attention with qh0 out-projection as filler
            emit_wot_dma(0)
            emit_wot_dma(1)
            fillB = []
            for itl in range(4):
                for ec in range(2):
                    fillB.append(lambda e=ec, i=itl: emit_outproj_unit(e, i))
            pumpB = mkpump(fillB)
            calls = [0]

            def pumpB_r(n):
                calls[0] += 1
                if calls[0] % 3 == 0:
                    pumpB(n)

            lrows1 = lp.tile([16, 512], F32, tag="lrows")
            for dqt in range(7):
                emit_pair_small(dqt, 1, lrows1, pumpB_r)
            emit_pair_big(1, lrows1, pumpB_r)
            emit_norm(1, lrows1)
            pumpB(len(fillB))  # leftovers cover norm1's serial chain
            # qh1 out-projection (tail; weights already resident)
            for itl in range(4, 8):
                for ec in range(2):
                    emit_outproj_unit(ec, itl)

    nc.compile()
    return nc


def _prep(x, ln_w, ln_b, Wq, Wk, Wv, Wo, M):
    x = np.asarray(x, np.float32)
    ln_w = np.asarray(ln_w, np.float32)
    ln_b = np.asarray(ln_b, np.float32)
    Wq = np.asarray(Wq, np.float32)
    Wk = np.asarray(Wk, np.float32)
    Wv = np.asarray(Wv, np.float32)
    Wo = np.asarray(Wo, np.float32)
    M = np.asarray(M, np.float32)
    assert not np.any(ln_b), "kernel assumes ln_b == 0"

    s_heads = (-M[:, 0, 1]).astype(np.float64)  # M[h,0,1] = -s_h
    Ts = [min(CTX, int(np.ceil(19.0 / s))) for s in s_heads]
    assert all(t <= 127 for t in Ts[:14]), "P3 pattern needs T<=127 for h0..13"

    wq_eff = ln_w[:, None] * Wq
    for h in range(NH):
        wq_eff[:, h * DH:(h + 1) * DH] /= 8.0 * s_heads[h]
    wk_eff = ln_w[:, None] * Wk
    wv_eff = ln_w[:, None] * Wv

    def wq_layout(w):  # [1024,1024] -> [dqt, p, ko, m]
        return np.ascontiguousarray(
            w.reshape(8, 128, 8, 128).transpose(2, 1, 0, 3)
        ).astype(ml_dtypes.bfloat16)

    wq_a = wq_layout(wq_eff)
    wk_a = wq_layout(wk_eff)
    wv_a = np.ascontiguousarray(wv_eff.reshape(8, 128, DIM)).astype(
        ml_dtypes.bfloat16
    )
    wo_a = np.ascontiguousarray(Wo.reshape(8, 128, DIM)).astype(ml_dtypes.bfloat16)

    ident = np.eye(128, dtype=np.float32).astype(ml_dtypes.bfloat16)
    oh = np.zeros((16, 2048), np.float32)
    for j in range(8):
        oh[2 * j, 128 * j:128 * j + 64] = 1.0
        oh[2 * j + 1, 128 * j + 64:128 * (j + 1)] = 1.0

    # master[pj, u]: r = u - pj - 2048 (= i_local - j_local)
    u = np.arange(3072, dtype=np.float64)[None, :]
    pj = np.arange(128, dtype=np.float64)[:, None]
    r = u - pj - 2048.0

    def _bf(a):
        return np.ascontiguousarray(
            np.maximum(a, -20000.0).astype(np.float32)
        ).astype(ml_dtypes.bfloat16)

    m0 = _bf(-np.abs(r[:, MTRIM:]))
    masters1 = [_bf(-np.abs(r[:, :2048])), _bf(-np.abs(r[:, :2048] + 2048.0))]

    in_maps = []
    for c in range(8):
        b, t = c // 2, c % 2
        xr = np.ascontiguousarray(np.roll(x[b], -QTOK * t, axis=0)).astype(
            ml_dtypes.bfloat16
        )
        in_maps.append({
            "x": xr, "wq": wq_a, "wk": wk_a, "wv": wv_a, "wo": wo_a,
            "master": m0, "master1": masters1[t], "ident": ident, "oh16": oh,
        })
    return s_heads, Ts, in_maps


def kernel(**inputs):
    global LAST_EXEC_NS
    s_heads, Ts, in_maps = _prep(**inputs)
    nc = _build_graph(s_heads, Ts)
    trace = os.environ.get("KERNEL_TRACE") == "1"
    res = run_bass_kernel_spmd(
        nc, in_maps, core_ids=list(range(8)), trace=trace
    )
    LAST_EXEC_NS = res.exec_time_ns
    out = np.empty((4, CTX, DIM), np.float32)
    for c in range(8):
        b, t = c // 2, c % 2
        out[b, QTOK * t:QTOK * (t + 1), :] = res.results[c]["out"]
    return out
